# revision 2
# baseline (speedup 1.0000x reference)
"""Trainium2 Bass kernel for nn_MAB_65068754534455 (dense transformer MAB block).

Computation (per reference):
  q = query @ Wq.T + bq ; k = kv @ Wk.T (bk dropped: softmax shift-invariant)
  v = kv @ Wv.T + bv
  per head: A = softmax(q k^T / 8) ; o = A v
  x = qheads + o (merged) ; out = x + relu(x @ Wo.T + bo)

Sharding: 8 cores = 4 batches x 2 query-halves (data parallel, no collectives).

v2 design (from microbenchmarks):
  - PE sustains 216ns per 512-moving matmul at 2.4GHz in continuous streams;
    row-group score pairs (64-contraction h0/h64) co-execute in one 216ns slot.
  - PV uses fp8 DoubleRow over KEY-TILE PAIRS: e8 tiles [128,2(par),2(head),512]
    f8; v8 f8 with ones cols. One DR matmul per (key pair, head) = 216ns,
    halving PV tensor time vs bf16.
  - exp outputs fp8 directly (scale C=1/8 cancels in the softmax normalize):
    scalar native Exp->f8 (bias=ln C), DVE Schraudolph u8 bit trick
    (bits = 1.4427*s + 31.655, saturating u8 store clamps the tails).
  - Engine-bound kernel: exp 16.8M elems across scalar+DVE ~ 77us floor.
    All SBUF-side work (softmax normalize mult/adds, phase3 residual adds)
    moves to GPSIMD (cannot read PSUM, but free capacity for SBUF ops).
  - K drains have no bias now; fungible scalar/DVE. o drains split
    scalar/DVE per block. V drains (psum->f8 + bv) on DVE stt.
"""

import math

import numpy as np
import ml_dtypes

import concourse.mybir as mybir
import concourse.tile as tile
from concourse import bacc
from concourse.bass_utils import run_bass_kernel_spmd

# problem constants (hardcoded per spec)
B, SQ, SKV, D, H = 4, 2048, 2048, 512, 8
HD = D // H                      # 64
SCALE = 1.0 / math.sqrt(HD)      # 1/8
NCORES = 8
TQ = SQ // 2                     # 1024 query rows per core

F32 = mybir.dt.float32
BF16 = mybir.dt.bfloat16
U8 = mybir.dt.uint8
F8 = mybir.dt.float8e4
W8SCALE = 64.0                   # host-side scale on fp8 K/V weights

KT = D // 128                    # 4 contraction k-tiles
DT = D // 128                    # 4 output d-tiles (== head pairs)
NQB = TQ // 512                  # 2 query blocks of 512
NKB = SKV // 512                 # 4 key blocks of 512
NTK = SKV // 128                 # 16 key tiles of 128
NKP = NTK // 2                   # 8 key-tile pairs
VW = HD + 2                      # 66: V head block [64 feats | ones | pad]
OC = HD + 1                      # 65 useful o rows (feats + r)

# fp8 exp scaling: e8 = exp(s*SCALE)/8; the 1/8 cancels in softmax normalize.
LN_C = math.log(1.0 / 8.0)
# DVE Schraudolph u8: bits = 8*(log2e*(s*SCALE) + 7 + log2C) - 8*0.0431
EXP_A8 = 8.0 / math.log(2.0) * SCALE          # 1.442695
EXP_B8 = 8.0 * (7.0 + math.log2(1 / 8.0)) - 8.0 * 0.0431   # 31.6552

# exp engine alternation is per KEY PAIR so each e8 tile has a single
# writer and (with e8 bufs=4) recycles to the same engine queue -- no
# cross-engine WAW semaphores. Block 0 leans scalar (DVE does V drains).
SCALAR_KPS = ([frozenset({0, 1, 2, 4, 6, 7}), frozenset({0, 2, 4, 6, 7}),
               frozenset({0, 2, 4, 6, 7})]
              + [frozenset({0, 2, 4, 6})] * 5)


def _build():
    nc = bacc.Bacc(None, target_bir_lowering=False, debug=False)

    xqt = nc.dram_tensor("xqt", [D, TQ], BF16, kind="ExternalInput").ap()
    xkvt = nc.dram_tensor("xkvt", [D, SKV], F8, kind="ExternalInput").ap()
    wqt = nc.dram_tensor("wqt", [D, D], BF16, kind="ExternalInput").ap()
    wkt = nc.dram_tensor("wkt", [D, D], F8, kind="ExternalInput").ap()
    wvt = nc.dram_tensor("wvt", [D, D], F8, kind="ExternalInput").ap()
    wot = nc.dram_tensor("wot", [D, D], BF16, kind="ExternalInput").ap()
    # biases in one tensor: [bq | bo | bv-broadcast]  (no bk)
    ball = nc.dram_tensor("ball", [128, 2 * DT + D], F32, kind="ExternalInput").ap()
    outt = nc.dram_tensor("outt", [D, TQ], BF16, kind="ExternalOutput").ap()

    with tile.TileContext(nc) as tc:
        with tc.tile_pool(name="persist", bufs=1) as pp:
            w_q = pp.tile([128, KT, D], BF16)
            w_k = pp.tile([128, KT, D], F8)
            w_v = pp.tile([128, KT, D], F8)
            w_o = pp.tile([128, KT, D], BF16)
            qt = pp.tile([128, DT, TQ], BF16)      # Q^T, becomes x^T
            kt = pp.tile([128, DT, SKV], BF16)     # K^T (scores lhsT)
            v8 = pp.tile([128, NTK, H * VW], F8)   # V f8 with ones cols
            xq_s = pp.tile([128, KT, TQ], BF16)
            xkv_s = pp.tile([128, KT, SKV], F8)
            b_s = pp.tile([128, 2 * DT + D], F32)
            bq_s = b_s[:, 0:DT]
            bo_s = b_s[:, DT : 2 * DT]
            bv_s = b_s[:, 2 * DT :]
            expw = pp.tile([128, 1], F32)
            zz = pp.tile([128, 1], F32)
            lnc = pp.tile([128, 1], F32)

            # ---- input DMAs spread across engine queues for parallel
            # enqueue + transfer; ordered so the Q-projection path lands
            # first ----
            wq_r = wqt.rearrange("(o p) d -> p o d", p=128)
            xq_r = xqt.rearrange("(o p) t -> p o t", p=128)
            xkv_r = xkvt.rearrange("(o p) t -> p o t", p=128)
            # critical prefix spread over the three DMA-capable queues:
            # scores(0,0,t=0) needs wq[j0]+xq[qb0]+bq and wk+xkv[kb0]
            nc.sync.dma_start(b_s[:, 0:DT], ball[:, 0:DT])          # bq (tiny)
            nc.scalar.dma_start(w_q[:, :, 0:128], wq_r[:, :, 0:128])
            nc.gpsimd.dma_start(w_k[:], wkt.rearrange("(o p) d -> p o d", p=128))
            nc.sync.dma_start(xq_s[:, :, 0:512], xq_r[:, :, 0:512])
            nc.scalar.dma_start(w_q[:, :, 128:512], wq_r[:, :, 128:512])
            nc.gpsimd.dma_start(xkv_s[:, :, 0:512], xkv_r[:, :, 0:512])
            nc.gpsimd.dma_start(w_v[:], wvt.rearrange("(o p) d -> p o d", p=128))
            nc.sync.dma_start(b_s[:, DT:], ball[:, DT:])            # bo | bv
            nc.scalar.dma_start(xkv_s[:, :, 1536:2048], xkv_r[:, :, 1536:2048])
            nc.gpsimd.dma_start(xkv_s[:, :, 512:1024], xkv_r[:, :, 512:1024])
            nc.gpsimd.dma_start(xkv_s[:, :, 1024:1536], xkv_r[:, :, 1024:1536])
            nc.sync.dma_start(xq_s[:, :, 512:1024], xq_r[:, :, 512:1024])
            nc.sync.dma_start(w_o[:], wot.rearrange("(o p) d -> p o d", p=128))

            # preload exp activation table early (no DMA dependency)
            nc.vector.memset(zz[:], 0.0)
            nc.vector.memset(lnc[:], LN_C)
            nc.scalar.activation(
                expw[:], zz[:], mybir.ActivationFunctionType.Exp
            )

            # ones columns of V (col 64 of each 65-wide head block)
            ones8 = pp.tile([128, H], F32)
            nc.gpsimd.memset(ones8[:], 1.0)
            for i in range(NTK):
                nc.gpsimd.tensor_copy(
                    v8[:, i, :].rearrange("p (h w) -> p h w", w=VW)[:, :, HD],
                    ones8[:],
                )

            with (
                tc.tile_pool(name="sp", bufs=3, space="PSUM") as sp,
                tc.tile_pool(name="op", bufs=1, space="PSUM") as opl,
                tc.tile_pool(name="e8", bufs=6) as e8p,
                tc.tile_pool(name="oc", bufs=2) as ocp,
                tc.tile_pool(name="rr", bufs=4) as rrp,
                tc.tile_pool(name="rb", bufs=4) as rbp,
                tc.tile_pool(name="on", bufs=4) as onp,
                tc.tile_pool(name="o64", bufs=2) as o64p,
                tc.tile_pool(name="yt", bufs=3) as yp,
            ):
                # ---------------- projections ----------------
                def q_proj(j, qb, drain="scalar"):
                    qsl = slice(qb * 512, (qb + 1) * 512)
                    ps = sp.tile([128, 2, 512], F32, tag="s2", name="s2t")
                    for k in range(KT):
                        nc.tensor.matmul(
                            ps[:, 0, :],
                            w_q[:, k, j * 128 : (j + 1) * 128],
                            xq_s[:, k, qsl],
                            start=(k == 0), stop=(k == KT - 1),
                        )
                    if drain == "scalar":
                        nc.scalar.activation(
                            qt[:, j, qsl], ps[:, 0, :],
                            mybir.ActivationFunctionType.Identity,
                            bias=bq_s[:, j : j + 1],
                        )
                    else:
                        nc.vector.tensor_scalar_add(
                            qt[:, j, qsl], ps[:, 0, :], bq_s[:, j : j + 1]
                        )

                def k_proj(j, c, drain="scalar"):
                    # K^T[j-tile, 512 keys] via fp8 DR (weights x64 on host,
                    # undone in the drain); no bias (softmax shift-invariant).
                    ksl = slice(c * 512, (c + 1) * 512)
                    ps = sp.tile([128, 2, 512], F32, tag="s2", name="s2t")
                    for g in range(KT // 2):
                        gs = slice(2 * g, 2 * g + 2)
                        nc.tensor.matmul(
                            ps[:, 0, :],
                            w_k[:, gs, j * 128 : (j + 1) * 128],
                            xkv_s[:, gs, ksl],
                            start=(g == 0), stop=(g == KT // 2 - 1),
                            perf_mode=mybir.MatmulPerfMode.DoubleRow,
                        )
                    if drain == "scalar":
                        nc.scalar.activation(
                            kt[:, j, ksl], ps[:, 0, :],
                            mybir.ActivationFunctionType.Copy,
                            scale=1.0 / W8SCALE,
                        )
                    else:
                        nc.vector.tensor_scalar_mul(
                            kt[:, j, ksl], ps[:, 0, :], 1.0 / W8SCALE
                        )

                def v_proj(blk):
                    # V natural [keys, feat] for one 128-key tile; f8 drain
                    # with bias on DVE.
                    isl = slice(blk * 128, (blk + 1) * 128)
                    ps = sp.tile([128, 2, 512], F32, tag="s2", name="s2t")
                    for g in range(KT // 2):
                        gs = slice(2 * g, 2 * g + 2)
                        nc.tensor.matmul(
                            ps[:, 0, :], xkv_s[:, gs, isl], w_v[:, gs, :],
                            start=(g == 0), stop=(g == KT // 2 - 1),
                            perf_mode=mybir.MatmulPerfMode.DoubleRow,
                        )
                    nc.vector.scalar_tensor_tensor(
                        v8[:, blk, :].rearrange("p (h w) -> p h w", w=VW)[:, :, 0:HD],
                        ps[:, 0, :].rearrange("p (h w) -> p h w", w=HD),
                        1.0 / W8SCALE,
                        bv_s.rearrange("p (h w) -> p h w", w=HD),
                        mybir.AluOpType.mult,
                        mybir.AluOpType.add,
                    )

                # ---------------- softmax normalize (deferred) ----------------
                pending_norm = []

                def norm_parity(hp, qb, oc, par):
                    # oc: SBUF copy of o psum [VW, 2, 512]; row 64 = r.
                    qsl = slice(qb * 512, (qb + 1) * 512)
                    rr2 = rrp.tile([1, 512], F32, name="rr2t")
                    nc.sync.dma_start(rr2[0:1, :], oc[64:65, par, :])
                    rr3 = rrp.tile([1, 512], F32, name="rr3t")
                    nc.vector.reciprocal_approx_fast(rr3[0:1, :], rr2[0:1, :])
                    rbc = rbp.tile([64, 512], F32)
                    nc.gpsimd.partition_broadcast(rbc[:], rr3[0:1, :])
                    on = onp.tile([64, 512], BF16)
                    nc.vector.tensor_tensor(
                        on[:], oc[0:64, par, :], rbc[:], mybir.AluOpType.mult
                    )
                    if par == 0:
                        nc.vector.tensor_tensor(
                            qt[0:64, hp, qsl], qt[0:64, hp, qsl], on[:],
                            mybir.AluOpType.add,
                        )
                    else:
                        on64 = o64p.tile([128, 512], BF16)
                        nc.sync.dma_start(on64[64:128, :], on[:])
                        nc.vector.tensor_tensor(
                            qt[64:128, hp, qsl], qt[64:128, hp, qsl],
                            on64[64:128, :], mybir.AluOpType.add,
                        )

                def flush_norm(idx):
                    while pending_norm and pending_norm[0][0] <= idx:
                        _, args = pending_norm.pop(0)
                        norm_parity(*args)

                # ---------------- attention ----------------
                def attn_block(hp, qb, extras=None, o_drain_eng="scalar",
                               pv_delay=3, tail_prev=None):
                    qsl = slice(qb * 512, (qb + 1) * 512)
                    h_e, h_o = 2 * hp, 2 * hp + 1
                    s_kps = SCALAR_KPS[2 * hp + qb]
                    o_e = opl.tile([VW, 512], F32, name="oe")
                    o_o = opl.tile([VW, 512], F32, name="oo")

                    e8_cur = None
                    e8_by_kp = {}

                    def pv(kp):
                        e8k = e8_by_kp.pop(kp)
                        ksl = slice(2 * kp, 2 * kp + 2)
                        nc.tensor.matmul(
                            o_e[:], v8[:, ksl, h_e * VW : (h_e + 1) * VW],
                            e8k[:, :, 0, :],
                            start=(kp == 0), stop=(kp == NKP - 1),
                            perf_mode=mybir.MatmulPerfMode.DoubleRow,
                        )
                        nc.tensor.matmul(
                            o_o[:], v8[:, ksl, h_o * VW : (h_o + 1) * VW],
                            e8k[:, :, 1, :],
                            start=(kp == 0), stop=(kp == NKP - 1),
                            perf_mode=mybir.MatmulPerfMode.DoubleRow,
                        )

                    for t in range(NTK):
                        flush_norm(t)
                        if extras and t in extras:
                            for fn in extras[t]:
                                fn()
                        isl = slice(t * 128, (t + 1) * 128)
                        s2 = sp.tile([128, 2, 512], F32, tag="s2", name="s2t")
                        nc.tensor.matmul(
                            s2[:, 0, :], kt[0:64, hp, isl], qt[0:64, hp, qsl],
                            start=True, stop=True,
                        )
                        nc.tensor.matmul(
                            s2[:, 1, :], kt[64:128, hp, isl], qt[64:128, hp, qsl],
                            start=True, stop=True,
                        )
                        kp, par = t // 2, t % 2
                        if par == 0:
                            e8_cur = e8p.tile([128, 2, 2, 512], F8, name="e8t")
                            e8_by_kp[kp] = e8_cur
                        if kp in s_kps:
                            nc.scalar.activation(
                                e8_cur[:, par, :, :], s2[:],
                                mybir.ActivationFunctionType.Exp,
                                bias=lnc[:, 0:1], scale=SCALE,
                            )
                        else:
                            nc.vector.tensor_scalar(
                                e8_cur[:, par, :, :].bitcast(U8), s2[:],
                                EXP_A8, EXP_B8,
                                mybir.AluOpType.mult, mybir.AluOpType.add,
                            )
                        # software pipeline: PV(kp) pv_delay iterations
                        # after its exps were issued
                        if t >= pv_delay and t % 2 == 1:
                            pv((t - pv_delay) // 2)

                    def tail():
                        # PV tail + o drain, deferred into the next block's
                        # second iteration so its first scores fill the PE
                        # while the last exps complete.
                        for kp in range((NTK - pv_delay) // 2 + 1, NKP):
                            pv(kp)
                        oc = ocp.tile([OC, 2, 512], F32, name="oct")
                        if o_drain_eng == "scalar":
                            nc.scalar.activation(
                                oc[:, 0, :], o_e[0:OC, :],
                                mybir.ActivationFunctionType.Copy
                            )
                            nc.vector.tensor_copy(oc[:, 1, :], o_o[0:OC, :])
                        else:
                            nc.vector.tensor_copy(oc[:, 0, :], o_e[0:OC, :])
                            nc.scalar.activation(
                                oc[:, 1, :], o_o[0:OC, :],
                                mybir.ActivationFunctionType.Copy
                            )
                        pending_norm.append((3, (hp, qb, oc, 0)))
                        pending_norm.append((8, (hp, qb, oc, 1)))
                    return tail

                # ---------------- phase 3 ----------------
                def phase3_mm(z, q, jj, ks):
                    qsl = slice(q * 512, (q + 1) * 512)
                    for half in range(2):
                        j = 2 * jj + half
                        for k in ks:
                            nc.tensor.matmul(
                                z[:, half, :],
                                w_o[:, k, j * 128 : (j + 1) * 128],
                                qt[:, k, qsl],
                                start=(k == 0), stop=(k == KT - 1),
                            )

                def phase3_drain(z, q, jj):
                    qsl = slice(q * 512, (q + 1) * 512)
                    for half in range(2):
                        j = 2 * jj + half
                        yt = yp.tile([128, 512], BF16, name="ytt")
                        nc.scalar.activation(
                            yt[:], z[:, half, :],
                            mybir.ActivationFunctionType.Relu,
                            bias=bo_s[:, j : j + 1],
                        )
                        yo = yp.tile([128, 512], BF16, name="yot")
                        nc.vector.tensor_tensor(
                            yo[:], yt[:], qt[:, j, qsl], mybir.AluOpType.add
                        )
                        nc.sync.dma_start(
                            outt[j * 128 : (j + 1) * 128, qsl], yo[:]
                        )

                # ---------------- schedule ----------------
                # minimal prelude, then block (0,0) self-feeds via extras
                # (PV delayed 5 iters there so JIT V projections keep up)
                q_proj(0, 0, drain="scalar")
                k_proj(0, 0, drain="scalar")
                ex00 = {
                    0: [lambda: q_proj(1, 0, "scalar")],
                    1: [lambda: q_proj(2, 0, "scalar")],
                    2: [lambda: q_proj(3, 0, "scalar"),
                        lambda: v_proj(0), lambda: v_proj(1)],
                    3: [lambda: v_proj(2), lambda: k_proj(0, 1, "scalar")],
                    4: [lambda: v_proj(3), lambda: v_proj(4)],
                    5: [lambda: v_proj(5)],
                    6: [lambda: v_proj(6), lambda: k_proj(0, 2, "scalar")],
                    7: [lambda: v_proj(7), lambda: v_proj(8)],
                    8: [lambda: v_proj(9)],
                    10: [lambda: v_proj(10), lambda: k_proj(0, 3, "scalar"),
                         lambda: v_proj(11)],
                    11: [lambda: v_proj(12)],
                    12: [lambda: v_proj(13)],
                    13: [lambda: v_proj(14), lambda: v_proj(15)],
                }
                attn_block(0, 0, extras=ex00, o_drain_eng="scalar",
                           pv_delay=5)()

                # Q(j0, qb1) must land before block (0,1) scores
                q_proj(0, 1, drain="scalar")
                ex01 = {
                    2: [lambda: k_proj(1, 0, "scalar")],
                    4: [lambda: q_proj(1, 1, "scalar"),
                        lambda: k_proj(1, 1, "scalar")],
                    7: [lambda: k_proj(1, 2, "scalar")],
                    9: [lambda: q_proj(2, 1, "scalar")],
                    11: [lambda: k_proj(1, 3, "scalar")],
                    13: [lambda: q_proj(3, 1, "scalar")],
                }
                attn_block(0, 1, extras=ex01, o_drain_eng="vector")()

                # K(j2) spread over blocks (1,0)/(1,1); K(j3) over (2,0)/(2,1)
                exk = lambda j, c, e: (lambda: k_proj(j, c, e))
                attn_block(1, 0, extras={
                    4: [exk(2, 0, "scalar")], 10: [exk(2, 1, "scalar")],
                }, o_drain_eng="scalar")()
                attn_block(1, 1, extras={
                    4: [exk(2, 2, "scalar")], 10: [exk(2, 3, "scalar")],
                }, o_drain_eng="vector")()
                attn_block(2, 0, extras={
                    4: [exk(3, 0, "scalar")], 10: [exk(3, 1, "scalar")],
                }, o_drain_eng="scalar")()
                attn_block(2, 1, extras={
                    4: [exk(3, 2, "scalar")], 10: [exk(3, 3, "scalar")],
                }, o_drain_eng="vector")()
                attn_block(3, 0, o_drain_eng="scalar")()
                attn_block(3, 1, o_drain_eng="vector")()

                # phase 3: qb0 first (norms already flushed); qb1
                # k=0..2 pre-flush, k=3 after the final norms.
                for jj in range(DT // 2):
                    z = sp.tile([128, 2, 512], F32, tag="s2", name="s2t")
                    phase3_mm(z, 0, jj, range(KT))
                    phase3_drain(z, 0, jj)
                z1 = [
                    sp.tile([128, 2, 512], F32, tag="s2", name="s2t")
                    for _ in range(DT // 2)
                ]
                for jj in range(DT // 2):
                    phase3_mm(z1[jj], 1, jj, range(KT - 1))
                flush_norm(NTK)
                for jj in range(DT // 2):
                    phase3_mm(z1[jj], 1, jj, [KT - 1])
                    phase3_drain(z1[jj], 1, jj)

    nc.compile()
    return nc


_NC = None


def _get_nc():
    global _NC
    if _NC is None:
        _NC = _build()
    return _NC


def kernel(**inputs) -> np.ndarray:
    bf = ml_dtypes.bfloat16
    f8 = ml_dtypes.float8_e4m3
    q = np.asarray(inputs["query"], dtype=np.float32)
    kv = np.asarray(inputs["key_value"], dtype=np.float32)
    shared = {
        "wqt": np.ascontiguousarray(np.asarray(inputs["Wq"], np.float32).T).astype(bf),
        "wkt": np.ascontiguousarray(np.asarray(inputs["Wk"], np.float32).T * W8SCALE).astype(f8),
        "wvt": np.ascontiguousarray(np.asarray(inputs["Wv"], np.float32).T * W8SCALE).astype(f8),
        "wot": np.ascontiguousarray(np.asarray(inputs["Wo"], np.float32).T).astype(bf),
        "ball": np.ascontiguousarray(np.concatenate(
            [
                np.asarray(inputs["bq"], np.float32).reshape(DT, 128).T,
                np.asarray(inputs["bo"], np.float32).reshape(DT, 128).T,
                np.broadcast_to(np.asarray(inputs["bv"], np.float32), (128, D)),
            ],
            axis=1,
        )),
    }
    in_maps = []
    for c in range(NCORES):
        b, half = divmod(c, 2)
        qs = q[b, half * TQ : (half + 1) * TQ]
        in_maps.append(
            {
                "xqt": np.ascontiguousarray(qs.T).astype(bf),
                "xkvt": np.ascontiguousarray(kv[b].T).astype(f8),
                **shared,
            }
        )

    nc = _get_nc()
    res = run_bass_kernel_spmd(nc, in_maps, core_ids=list(range(NCORES)))
    kernel._last_results = res  # for test harness introspection

    out = np.empty((B, SQ, D), np.float32)
    for c in range(NCORES):
        b, half = divmod(c, 2)
        out[b, half * TQ : (half + 1) * TQ] = res.results[c]["outt"].astype(np.float32).T
    return out


# revision 3
# speedup vs baseline: 1.0020x; 1.0020x over previous
"""Trainium2 Bass kernel for nn_MAB_65068754534455 (dense transformer MAB block).

Computation (per reference):
  q = query @ Wq.T + bq ; k = kv @ Wk.T (bk dropped: softmax shift-invariant)
  v = kv @ Wv.T + bv
  per head: A = softmax(q k^T / 8) ; o = A v
  x = qheads + o (merged) ; out = x + relu(x @ Wo.T + bo)

Sharding: 8 cores = 4 batches x 2 query-halves (data parallel, no collectives).

v2 design (from microbenchmarks):
  - PE sustains 216ns per 512-moving matmul at 2.4GHz in continuous streams;
    row-group score pairs (64-contraction h0/h64) co-execute in one 216ns slot.
  - PV uses fp8 DoubleRow over KEY-TILE PAIRS: e8 tiles [128,2(par),2(head),512]
    f8; v8 f8 with ones cols. One DR matmul per (key pair, head) = 216ns,
    halving PV tensor time vs bf16.
  - exp outputs fp8 directly (scale C=1/8 cancels in the softmax normalize):
    scalar native Exp->f8 (bias=ln C), DVE Schraudolph u8 bit trick
    (bits = 1.4427*s + 31.655, saturating u8 store clamps the tails).
  - Engine-bound kernel: exp 16.8M elems across scalar+DVE ~ 77us floor.
    exp alternates engines per KEY PAIR so each e8 tile has one writer and
    (bufs=6, even) recycles onto the same engine queue -- standalone
    cross-engine WAW waits were the dominant failure mode of earlier drafts.
  - GPSIMD does ONLY attn-lib partition_broadcasts: mixing its standard-lib
    tensor ops with bcasts forces per-chain library swaps that serialize the
    whole pipeline (+150us measured). Norm mult/add/recip, V drains, and
    phase3 residual adds therefore stay on DVE; K/Q drains, o-drain halves,
    and phase3 relu on scalar.
  - K drains have no bias (bk provably cancels in softmax). Input DMAs are
    split by first-use and spread over the sync/scalar/gpsimd queues; block
    (0,0) starts ~13.5us in and self-feeds Q/K/V projections via extras
    with PV delayed 5 iterations.
  - Measured: 178-184us (run-to-run device variance up to +-17% across
    processes; within-process spread ~0.5%). Baseline was 221us.
"""

import math

import numpy as np
import ml_dtypes

import concourse.mybir as mybir
import concourse.tile as tile
from concourse import bacc
from concourse.bass_utils import run_bass_kernel_spmd

# problem constants (hardcoded per spec)
B, SQ, SKV, D, H = 4, 2048, 2048, 512, 8
HD = D // H                      # 64
SCALE = 1.0 / math.sqrt(HD)      # 1/8
NCORES = 8
TQ = SQ // 2                     # 1024 query rows per core

F32 = mybir.dt.float32
BF16 = mybir.dt.bfloat16
U8 = mybir.dt.uint8
F8 = mybir.dt.float8e4
W8SCALE = 64.0                   # host-side scale on fp8 K/V weights

KT = D // 128                    # 4 contraction k-tiles
DT = D // 128                    # 4 output d-tiles (== head pairs)
NQB = TQ // 512                  # 2 query blocks of 512
NKB = SKV // 512                 # 4 key blocks of 512
NTK = SKV // 128                 # 16 key tiles of 128
NKP = NTK // 2                   # 8 key-tile pairs
VW = HD + 2                      # 66: V head block [64 feats | ones | pad]
OC = HD + 1                      # 65 useful o rows (feats + r)

# fp8 exp scaling: e8 = exp(s*SCALE)/8; the 1/8 cancels in softmax normalize.
LN_C = math.log(1.0 / 8.0)
# DVE Schraudolph u8: bits = 8*(log2e*(s*SCALE) + 7 + log2C) - 8*0.0431
EXP_A8 = 8.0 / math.log(2.0) * SCALE          # 1.442695
EXP_B8 = 8.0 * (7.0 + math.log2(1 / 8.0)) - 8.0 * 0.0431   # 31.6552

# exp engine alternation is per KEY PAIR so each e8 tile has a single
# writer and (with e8 bufs=4) recycles to the same engine queue -- no
# cross-engine WAW semaphores. Block 0 leans scalar (DVE does V drains).
SCALAR_KPS = ([frozenset({0, 1, 2, 4, 6, 7}), frozenset({0, 2, 4, 6, 7}),
               frozenset({0, 2, 4, 6, 7})]
              + [frozenset({0, 2, 4, 6})] * 5)


def _build():
    nc = bacc.Bacc(None, target_bir_lowering=False, debug=False)

    xqt = nc.dram_tensor("xqt", [D, TQ], BF16, kind="ExternalInput").ap()
    xkvt = nc.dram_tensor("xkvt", [D, SKV], F8, kind="ExternalInput").ap()
    wqt = nc.dram_tensor("wqt", [D, D], BF16, kind="ExternalInput").ap()
    wkt = nc.dram_tensor("wkt", [D, D], F8, kind="ExternalInput").ap()
    wvt = nc.dram_tensor("wvt", [D, D], F8, kind="ExternalInput").ap()
    wot = nc.dram_tensor("wot", [D, D], BF16, kind="ExternalInput").ap()
    # biases in one tensor: [bq | bo | bv-broadcast]  (no bk)
    ball = nc.dram_tensor("ball", [128, 2 * DT + D], F32, kind="ExternalInput").ap()
    outt = nc.dram_tensor("outt", [D, TQ], BF16, kind="ExternalOutput").ap()

    with tile.TileContext(nc) as tc:
        with tc.tile_pool(name="persist", bufs=1) as pp:
            w_q = pp.tile([128, KT, D], BF16)
            w_k = pp.tile([128, KT, D], F8)
            w_v = pp.tile([128, KT, D], F8)
            w_o = pp.tile([128, KT, D], BF16)
            qt = pp.tile([128, DT, TQ], BF16)      # Q^T, becomes x^T
            kt = pp.tile([128, DT, SKV], BF16)     # K^T (scores lhsT)
            v8 = pp.tile([128, NTK, H * VW], F8)   # V f8 with ones cols
            xq_s = pp.tile([128, KT, TQ], BF16)
            xkv_s = pp.tile([128, KT, SKV], F8)
            b_s = pp.tile([128, 2 * DT + D], F32)
            bq_s = b_s[:, 0:DT]
            bo_s = b_s[:, DT : 2 * DT]
            bv_s = b_s[:, 2 * DT :]
            expw = pp.tile([128, 1], F32)
            zz = pp.tile([128, 1], F32)
            lnc = pp.tile([128, 1], F32)

            # ---- input DMAs spread across engine queues for parallel
            # enqueue + transfer; ordered so the Q-projection path lands
            # first ----
            wq_r = wqt.rearrange("(o p) d -> p o d", p=128)
            xq_r = xqt.rearrange("(o p) t -> p o t", p=128)
            xkv_r = xkvt.rearrange("(o p) t -> p o t", p=128)
            # critical prefix spread over the three DMA-capable queues:
            # scores(0,0,t=0) needs wq[j0]+xq[qb0]+bq and wk+xkv[kb0]
            nc.sync.dma_start(b_s[:, 0:DT], ball[:, 0:DT])          # bq (tiny)
            nc.scalar.dma_start(w_q[:, :, 0:128], wq_r[:, :, 0:128])
            nc.gpsimd.dma_start(w_k[:], wkt.rearrange("(o p) d -> p o d", p=128))
            nc.sync.dma_start(xq_s[:, :, 0:512], xq_r[:, :, 0:512])
            nc.scalar.dma_start(w_q[:, :, 128:512], wq_r[:, :, 128:512])
            nc.gpsimd.dma_start(xkv_s[:, :, 0:512], xkv_r[:, :, 0:512])
            nc.gpsimd.dma_start(w_v[:], wvt.rearrange("(o p) d -> p o d", p=128))
            nc.sync.dma_start(b_s[:, DT:], ball[:, DT:])            # bo | bv
            nc.scalar.dma_start(xkv_s[:, :, 1536:2048], xkv_r[:, :, 1536:2048])
            nc.gpsimd.dma_start(xkv_s[:, :, 512:1024], xkv_r[:, :, 512:1024])
            nc.gpsimd.dma_start(xkv_s[:, :, 1024:1536], xkv_r[:, :, 1024:1536])
            nc.sync.dma_start(xq_s[:, :, 512:1024], xq_r[:, :, 512:1024])
            nc.sync.dma_start(w_o[:], wot.rearrange("(o p) d -> p o d", p=128))

            # preload exp activation table early (no DMA dependency)
            nc.vector.memset(zz[:], 0.0)
            nc.vector.memset(lnc[:], LN_C)
            nc.scalar.activation(
                expw[:], zz[:], mybir.ActivationFunctionType.Exp
            )

            # ones columns of V (col 64 of each 65-wide head block)
            ones8 = pp.tile([128, H], F32)
            nc.gpsimd.memset(ones8[:], 1.0)
            for i in range(NTK):
                nc.gpsimd.tensor_copy(
                    v8[:, i, :].rearrange("p (h w) -> p h w", w=VW)[:, :, HD],
                    ones8[:],
                )

            with (
                tc.tile_pool(name="sp", bufs=3, space="PSUM") as sp,
                tc.tile_pool(name="op", bufs=1, space="PSUM") as opl,
                tc.tile_pool(name="e8", bufs=6) as e8p,
                tc.tile_pool(name="oc", bufs=2) as ocp,
                tc.tile_pool(name="rr", bufs=4) as rrp,
                tc.tile_pool(name="rb", bufs=4) as rbp,
                tc.tile_pool(name="on", bufs=4) as onp,
                tc.tile_pool(name="o64", bufs=2) as o64p,
                tc.tile_pool(name="yt", bufs=3) as yp,
            ):
                # ---------------- projections ----------------
                def q_proj(j, qb, drain="scalar"):
                    qsl = slice(qb * 512, (qb + 1) * 512)
                    ps = sp.tile([128, 2, 512], F32, tag="s2", name="s2t")
                    for k in range(KT):
                        nc.tensor.matmul(
                            ps[:, 0, :],
                            w_q[:, k, j * 128 : (j + 1) * 128],
                            xq_s[:, k, qsl],
                            start=(k == 0), stop=(k == KT - 1),
                        )
                    if drain == "scalar":
                        nc.scalar.activation(
                            qt[:, j, qsl], ps[:, 0, :],
                            mybir.ActivationFunctionType.Identity,
                            bias=bq_s[:, j : j + 1],
                        )
                    else:
                        nc.vector.tensor_scalar_add(
                            qt[:, j, qsl], ps[:, 0, :], bq_s[:, j : j + 1]
                        )

                def k_proj(j, c, drain="scalar"):
                    # K^T[j-tile, 512 keys] via fp8 DR (weights x64 on host,
                    # undone in the drain); no bias (softmax shift-invariant).
                    ksl = slice(c * 512, (c + 1) * 512)
                    ps = sp.tile([128, 2, 512], F32, tag="s2", name="s2t")
                    for g in range(KT // 2):
                        gs = slice(2 * g, 2 * g + 2)
                        nc.tensor.matmul(
                            ps[:, 0, :],
                            w_k[:, gs, j * 128 : (j + 1) * 128],
                            xkv_s[:, gs, ksl],
                            start=(g == 0), stop=(g == KT // 2 - 1),
                            perf_mode=mybir.MatmulPerfMode.DoubleRow,
                        )
                    if drain == "scalar":
                        nc.scalar.activation(
                            kt[:, j, ksl], ps[:, 0, :],
                            mybir.ActivationFunctionType.Copy,
                            scale=1.0 / W8SCALE,
                        )
                    else:
                        nc.vector.tensor_scalar_mul(
                            kt[:, j, ksl], ps[:, 0, :], 1.0 / W8SCALE
                        )

                def v_proj(blk):
                    # V natural [keys, feat] for one 128-key tile; f8 drain
                    # with bias on DVE.
                    isl = slice(blk * 128, (blk + 1) * 128)
                    ps = sp.tile([128, 2, 512], F32, tag="s2", name="s2t")
                    for g in range(KT // 2):
                        gs = slice(2 * g, 2 * g + 2)
                        nc.tensor.matmul(
                            ps[:, 0, :], xkv_s[:, gs, isl], w_v[:, gs, :],
                            start=(g == 0), stop=(g == KT // 2 - 1),
                            perf_mode=mybir.MatmulPerfMode.DoubleRow,
                        )
                    nc.vector.scalar_tensor_tensor(
                        v8[:, blk, :].rearrange("p (h w) -> p h w", w=VW)[:, :, 0:HD],
                        ps[:, 0, :].rearrange("p (h w) -> p h w", w=HD),
                        1.0 / W8SCALE,
                        bv_s.rearrange("p (h w) -> p h w", w=HD),
                        mybir.AluOpType.mult,
                        mybir.AluOpType.add,
                    )

                # ---------------- softmax normalize (deferred) ----------------
                pending_norm = []

                def norm_parity(hp, qb, oc, par):
                    # oc: SBUF copy of o psum [VW, 2, 512]; row 64 = r.
                    qsl = slice(qb * 512, (qb + 1) * 512)
                    rr2 = rrp.tile([1, 512], F32, name="rr2t")
                    nc.sync.dma_start(rr2[0:1, :], oc[64:65, par, :])
                    rr3 = rrp.tile([1, 512], F32, name="rr3t")
                    nc.vector.reciprocal_approx_fast(rr3[0:1, :], rr2[0:1, :])
                    rbc = rbp.tile([64, 512], F32)
                    nc.gpsimd.partition_broadcast(rbc[:], rr3[0:1, :])
                    on = onp.tile([64, 512], BF16)
                    nc.vector.tensor_tensor(
                        on[:], oc[0:64, par, :], rbc[:], mybir.AluOpType.mult
                    )
                    if par == 0:
                        nc.vector.tensor_tensor(
                            qt[0:64, hp, qsl], qt[0:64, hp, qsl], on[:],
                            mybir.AluOpType.add,
                        )
                    else:
                        on64 = o64p.tile([128, 512], BF16)
                        nc.sync.dma_start(on64[64:128, :], on[:])
                        nc.vector.tensor_tensor(
                            qt[64:128, hp, qsl], qt[64:128, hp, qsl],
                            on64[64:128, :], mybir.AluOpType.add,
                        )

                def flush_norm(idx):
                    while pending_norm and pending_norm[0][0] <= idx:
                        _, args = pending_norm.pop(0)
                        norm_parity(*args)

                # ---------------- attention ----------------
                def attn_block(hp, qb, extras=None, o_drain_eng="scalar",
                               pv_delay=3, tail_prev=None):
                    qsl = slice(qb * 512, (qb + 1) * 512)
                    h_e, h_o = 2 * hp, 2 * hp + 1
                    s_kps = SCALAR_KPS[2 * hp + qb]
                    o_e = opl.tile([VW, 512], F32, name="oe")
                    o_o = opl.tile([VW, 512], F32, name="oo")

                    e8_cur = None
                    e8_by_kp = {}

                    def pv(kp):
                        e8k = e8_by_kp.pop(kp)
                        ksl = slice(2 * kp, 2 * kp + 2)
                        nc.tensor.matmul(
                            o_e[:], v8[:, ksl, h_e * VW : (h_e + 1) * VW],
                            e8k[:, :, 0, :],
                            start=(kp == 0), stop=(kp == NKP - 1),
                            perf_mode=mybir.MatmulPerfMode.DoubleRow,
                        )
                        nc.tensor.matmul(
                            o_o[:], v8[:, ksl, h_o * VW : (h_o + 1) * VW],
                            e8k[:, :, 1, :],
                            start=(kp == 0), stop=(kp == NKP - 1),
                            perf_mode=mybir.MatmulPerfMode.DoubleRow,
                        )

                    for t in range(NTK):
                        flush_norm(t)
                        if extras and t in extras:
                            for fn in extras[t]:
                                fn()
                        isl = slice(t * 128, (t + 1) * 128)
                        s2 = sp.tile([128, 2, 512], F32, tag="s2", name="s2t")
                        nc.tensor.matmul(
                            s2[:, 0, :], kt[0:64, hp, isl], qt[0:64, hp, qsl],
                            start=True, stop=True,
                        )
                        nc.tensor.matmul(
                            s2[:, 1, :], kt[64:128, hp, isl], qt[64:128, hp, qsl],
                            start=True, stop=True,
                        )
                        kp, par = t // 2, t % 2
                        if par == 0:
                            e8_cur = e8p.tile([128, 2, 2, 512], F8, name="e8t")
                            e8_by_kp[kp] = e8_cur
                        if kp in s_kps:
                            nc.scalar.activation(
                                e8_cur[:, par, :, :], s2[:],
                                mybir.ActivationFunctionType.Exp,
                                bias=lnc[:, 0:1], scale=SCALE,
                            )
                        else:
                            nc.vector.tensor_scalar(
                                e8_cur[:, par, :, :].bitcast(U8), s2[:],
                                EXP_A8, EXP_B8,
                                mybir.AluOpType.mult, mybir.AluOpType.add,
                            )
                        # software pipeline: PV(kp) pv_delay iterations
                        # after its exps were issued
                        if t >= pv_delay and t % 2 == 1:
                            pv((t - pv_delay) // 2)

                    def tail():
                        # PV tail + o drain, deferred into the next block's
                        # second iteration so its first scores fill the PE
                        # while the last exps complete.
                        for kp in range((NTK - pv_delay) // 2 + 1, NKP):
                            pv(kp)
                        oc = ocp.tile([OC, 2, 512], F32, name="oct")
                        if o_drain_eng == "scalar":
                            nc.scalar.activation(
                                oc[:, 0, :], o_e[0:OC, :],
                                mybir.ActivationFunctionType.Copy
                            )
                            nc.vector.tensor_copy(oc[:, 1, :], o_o[0:OC, :])
                        else:
                            nc.vector.tensor_copy(oc[:, 0, :], o_e[0:OC, :])
                            nc.scalar.activation(
                                oc[:, 1, :], o_o[0:OC, :],
                                mybir.ActivationFunctionType.Copy
                            )
                        pending_norm.append((3, (hp, qb, oc, 0)))
                        pending_norm.append((8, (hp, qb, oc, 1)))
                    return tail

                # ---------------- phase 3 ----------------
                def phase3_mm(z, q, jj, ks):
                    qsl = slice(q * 512, (q + 1) * 512)
                    for half in range(2):
                        j = 2 * jj + half
                        for k in ks:
                            nc.tensor.matmul(
                                z[:, half, :],
                                w_o[:, k, j * 128 : (j + 1) * 128],
                                qt[:, k, qsl],
                                start=(k == 0), stop=(k == KT - 1),
                            )

                def phase3_drain(z, q, jj):
                    qsl = slice(q * 512, (q + 1) * 512)
                    for half in range(2):
                        j = 2 * jj + half
                        yt = yp.tile([128, 512], BF16, name="ytt")
                        nc.scalar.activation(
                            yt[:], z[:, half, :],
                            mybir.ActivationFunctionType.Relu,
                            bias=bo_s[:, j : j + 1],
                        )
                        yo = yp.tile([128, 512], BF16, name="yot")
                        nc.vector.tensor_tensor(
                            yo[:], yt[:], qt[:, j, qsl], mybir.AluOpType.add
                        )
                        nc.sync.dma_start(
                            outt[j * 128 : (j + 1) * 128, qsl], yo[:]
                        )

                # ---------------- schedule ----------------
                # minimal prelude, then block (0,0) self-feeds via extras
                # (PV delayed 5 iters there so JIT V projections keep up)
                q_proj(0, 0, drain="scalar")
                k_proj(0, 0, drain="scalar")
                ex00 = {
                    0: [lambda: q_proj(1, 0, "scalar")],
                    1: [lambda: q_proj(2, 0, "scalar")],
                    2: [lambda: q_proj(3, 0, "scalar"),
                        lambda: v_proj(0), lambda: v_proj(1)],
                    3: [lambda: v_proj(2), lambda: k_proj(0, 1, "scalar")],
                    4: [lambda: v_proj(3), lambda: v_proj(4)],
                    5: [lambda: v_proj(5)],
                    6: [lambda: v_proj(6), lambda: k_proj(0, 2, "scalar")],
                    7: [lambda: v_proj(7), lambda: v_proj(8)],
                    8: [lambda: v_proj(9)],
                    10: [lambda: v_proj(10), lambda: k_proj(0, 3, "scalar"),
                         lambda: v_proj(11)],
                    11: [lambda: v_proj(12)],
                    12: [lambda: v_proj(13)],
                    13: [lambda: v_proj(14), lambda: v_proj(15)],
                }
                attn_block(0, 0, extras=ex00, o_drain_eng="scalar",
                           pv_delay=5)()

                # Q(j0, qb1) must land before block (0,1) scores
                q_proj(0, 1, drain="scalar")
                ex01 = {
                    2: [lambda: k_proj(1, 0, "scalar")],
                    4: [lambda: q_proj(1, 1, "scalar"),
                        lambda: k_proj(1, 1, "scalar")],
                    7: [lambda: k_proj(1, 2, "scalar")],
                    9: [lambda: q_proj(2, 1, "scalar")],
                    11: [lambda: k_proj(1, 3, "scalar")],
                    13: [lambda: q_proj(3, 1, "scalar")],
                }
                attn_block(0, 1, extras=ex01, o_drain_eng="vector")()

                # K(j2) spread over blocks (1,0)/(1,1); K(j3) over (2,0)/(2,1)
                exk = lambda j, c, e: (lambda: k_proj(j, c, e))
                attn_block(1, 0, extras={
                    4: [exk(2, 0, "scalar")], 10: [exk(2, 1, "scalar")],
                }, o_drain_eng="scalar")()
                attn_block(1, 1, extras={
                    4: [exk(2, 2, "scalar")], 10: [exk(2, 3, "scalar")],
                }, o_drain_eng="vector")()
                attn_block(2, 0, extras={
                    4: [exk(3, 0, "scalar")], 10: [exk(3, 1, "scalar")],
                }, o_drain_eng="scalar")()
                attn_block(2, 1, extras={
                    4: [exk(3, 2, "scalar")], 10: [exk(3, 3, "scalar")],
                }, o_drain_eng="vector")()
                attn_block(3, 0, o_drain_eng="scalar")()
                attn_block(3, 1, o_drain_eng="vector")()

                # phase 3: qb0 first (norms already flushed); qb1
                # k=0..2 pre-flush, k=3 after the final norms.
                for jj in range(DT // 2):
                    z = sp.tile([128, 2, 512], F32, tag="s2", name="s2t")
                    phase3_mm(z, 0, jj, range(KT))
                    phase3_drain(z, 0, jj)
                z1 = [
                    sp.tile([128, 2, 512], F32, tag="s2", name="s2t")
                    for _ in range(DT // 2)
                ]
                for jj in range(DT // 2):
                    phase3_mm(z1[jj], 1, jj, range(KT - 1))
                flush_norm(NTK)
                for jj in range(DT // 2):
                    phase3_mm(z1[jj], 1, jj, [KT - 1])
                    phase3_drain(z1[jj], 1, jj)

    nc.compile()
    return nc


_NC = None


def _get_nc():
    global _NC
    if _NC is None:
        _NC = _build()
    return _NC


def kernel(**inputs) -> np.ndarray:
    bf = ml_dtypes.bfloat16
    f8 = ml_dtypes.float8_e4m3
    q = np.asarray(inputs["query"], dtype=np.float32)
    kv = np.asarray(inputs["key_value"], dtype=np.float32)
    shared = {
        "wqt": np.ascontiguousarray(np.asarray(inputs["Wq"], np.float32).T).astype(bf),
        "wkt": np.ascontiguousarray(np.asarray(inputs["Wk"], np.float32).T * W8SCALE).astype(f8),
        "wvt": np.ascontiguousarray(np.asarray(inputs["Wv"], np.float32).T * W8SCALE).astype(f8),
        "wot": np.ascontiguousarray(np.asarray(inputs["Wo"], np.float32).T).astype(bf),
        "ball": np.ascontiguousarray(np.concatenate(
            [
                np.asarray(inputs["bq"], np.float32).reshape(DT, 128).T,
                np.asarray(inputs["bo"], np.float32).reshape(DT, 128).T,
                np.broadcast_to(np.asarray(inputs["bv"], np.float32), (128, D)),
            ],
            axis=1,
        )),
    }
    in_maps = []
    for c in range(NCORES):
        b, half = divmod(c, 2)
        qs = q[b, half * TQ : (half + 1) * TQ]
        in_maps.append(
            {
                "xqt": np.ascontiguousarray(qs.T).astype(bf),
                "xkvt": np.ascontiguousarray(kv[b].T).astype(f8),
                **shared,
            }
        )

    nc = _get_nc()
    res = run_bass_kernel_spmd(nc, in_maps, core_ids=list(range(NCORES)))
    kernel._last_results = res  # for test harness introspection

    out = np.empty((B, SQ, D), np.float32)
    for c in range(NCORES):
        b, half = divmod(c, 2)
        out[b, half * TQ : (half + 1) * TQ] = res.results[c]["outt"].astype(np.float32).T
    return out


# revision 5
# speedup vs baseline: 1.0211x; 1.0191x over previous
"""Trainium2 Bass kernel for nn_MAB_65068754534455 (dense transformer MAB block).

Computation (per reference):
  q = query @ Wq.T + bq ; k = kv @ Wk.T (bk dropped: softmax shift-invariant)
  v = kv @ Wv.T + bv
  per head: A = softmax(q k^T / 8) ; o = A v
  x = qheads + o (merged) ; out = x + relu(x @ Wo.T + bo)

Sharding: 8 cores = 4 batches x 2 query-halves (data parallel, no collectives).

v2 design (from microbenchmarks):
  - PE sustains 216ns per 512-moving matmul at 2.4GHz in continuous streams;
    row-group score pairs (64-contraction h0/h64) co-execute in one 216ns slot.
  - PV uses fp8 DoubleRow over KEY-TILE PAIRS: e8 tiles [128,2(par),2(head),512]
    f8; v8 f8 with ones cols. One DR matmul per (key pair, head) = 216ns,
    halving PV tensor time vs bf16.
  - exp outputs fp8 directly (scale C=1/8 cancels in the softmax normalize):
    scalar native Exp->f8 (bias=ln C), DVE Schraudolph u8 bit trick
    (bits = 1.4427*s + 31.655, saturating u8 store clamps the tails).
  - Engine-bound kernel: exp 16.8M elems across scalar+DVE ~ 77us floor.
    exp alternates engines per KEY PAIR so each e8 tile has one writer and
    (bufs=6, even) recycles onto the same engine queue -- standalone
    cross-engine WAW waits were the dominant failure mode of earlier drafts.
  - GPSIMD does ONLY attn-lib partition_broadcasts: mixing its standard-lib
    tensor ops with bcasts forces per-chain library swaps that serialize the
    whole pipeline (+150us measured). Norm mult/add/recip, V drains, and
    phase3 residual adds therefore stay on DVE; K/Q drains, o-drain halves,
    and phase3 relu on scalar.
  - K drains have no bias (bk provably cancels in softmax). Input DMAs are
    split by first-use and spread over the sync/scalar/gpsimd queues; block
    (0,0) starts ~13.5us in and self-feeds Q/K/V projections via extras
    with PV delayed 5 iterations.
  - Measured: 178-184us (run-to-run device variance up to +-17% across
    processes; within-process spread ~0.5%). Baseline was 221us.
"""

import math

import numpy as np
import ml_dtypes

import concourse.mybir as mybir
import concourse.tile as tile
from concourse import bacc
from concourse.bass_utils import run_bass_kernel_spmd

# problem constants (hardcoded per spec)
B, SQ, SKV, D, H = 4, 2048, 2048, 512, 8
HD = D // H                      # 64
SCALE = 1.0 / math.sqrt(HD)      # 1/8
NCORES = 8
TQ = SQ // 2                     # 1024 query rows per core

F32 = mybir.dt.float32
BF16 = mybir.dt.bfloat16
U8 = mybir.dt.uint8
F8 = mybir.dt.float8e4
W8SCALE = 64.0                   # host-side scale on fp8 K/V weights

KT = D // 128                    # 4 contraction k-tiles
DT = D // 128                    # 4 output d-tiles (== head pairs)
NQB = TQ // 512                  # 2 query blocks of 512
NKB = SKV // 512                 # 4 key blocks of 512
NTK = SKV // 128                 # 16 key tiles of 128
NKP = NTK // 2                   # 8 key-tile pairs
VW = HD + 2                      # 66: V head block [64 feats | ones | pad]
OC = HD + 1                      # 65 useful o rows (feats + r)

# fp8 exp scaling: e8 = exp(s*SCALE)/8; the 1/8 cancels in softmax normalize.
LN_C = math.log(1.0 / 8.0)
# DVE Schraudolph u8: bits = 8*(log2e*(s*SCALE) + 7 + log2C) - 8*0.0431
EXP_A8 = 8.0 / math.log(2.0) * SCALE          # 1.442695
EXP_B8 = 8.0 * (7.0 + math.log2(1 / 8.0)) - 8.0 * 0.0431   # 31.6552

# exp engine alternation is per KEY PAIR so each e8 tile has a single
# writer and (with e8 bufs=4) recycles to the same engine queue -- no
# cross-engine WAW semaphores. Block 0 leans scalar (DVE does V drains).
PV_DELAY = 5          # in-loop PV lag (iterations): more exp->PV slack
O_BOTH_SCALAR = True  # both o-drain halves on scalar (DVE is the pacer)
SCALAR_KPS = ([frozenset({0, 1, 2, 4, 6, 7}), frozenset({0, 2, 4, 6, 7}),
               frozenset({0, 2, 4, 6, 7})]
              + [frozenset({0, 2, 4, 6})] * 5)


def _build():
    nc = bacc.Bacc(None, target_bir_lowering=False, debug=False)

    xqt = nc.dram_tensor("xqt", [D, TQ], BF16, kind="ExternalInput").ap()
    xkvt = nc.dram_tensor("xkvt", [D, SKV], F8, kind="ExternalInput").ap()
    wqt = nc.dram_tensor("wqt", [D, D], BF16, kind="ExternalInput").ap()
    wkt = nc.dram_tensor("wkt", [D, D], F8, kind="ExternalInput").ap()
    wvt = nc.dram_tensor("wvt", [D, D], F8, kind="ExternalInput").ap()
    wot = nc.dram_tensor("wot", [D, D], BF16, kind="ExternalInput").ap()
    # biases in one tensor: [bq | bo | bv-broadcast]  (no bk)
    ball = nc.dram_tensor("ball", [128, 2 * DT + D], F32, kind="ExternalInput").ap()
    outt = nc.dram_tensor("outt", [D, TQ], BF16, kind="ExternalOutput").ap()

    with tile.TileContext(nc) as tc:
        with tc.tile_pool(name="persist", bufs=1) as pp:
            w_q = pp.tile([128, KT, D], BF16)
            w_k = pp.tile([128, KT, D], F8)
            w_v = pp.tile([128, KT, D], F8)
            w_o = pp.tile([128, KT, D], BF16)
            qt = pp.tile([128, DT, TQ], BF16)      # Q^T, becomes x^T
            kt = pp.tile([128, DT, SKV], BF16)     # K^T (scores lhsT)
            v8 = pp.tile([128, NTK, H * VW], F8)   # V f8 with ones cols
            xq_s = pp.tile([128, KT, TQ], BF16)
            xkv_s = pp.tile([128, KT, SKV], F8)
            b_s = pp.tile([128, 2 * DT + D], F32)
            bq_s = b_s[:, 0:DT]
            bo_s = b_s[:, DT : 2 * DT]
            bv_s = b_s[:, 2 * DT :]
            expw = pp.tile([128, 1], F32)
            zz = pp.tile([128, 1], F32)
            lnc = pp.tile([128, 1], F32)

            # ---- input DMAs spread across engine queues for parallel
            # enqueue + transfer; ordered so the Q-projection path lands
            # first ----
            wq_r = wqt.rearrange("(o p) d -> p o d", p=128)
            xq_r = xqt.rearrange("(o p) t -> p o t", p=128)
            xkv_r = xkvt.rearrange("(o p) t -> p o t", p=128)
            # critical prefix spread over the three DMA-capable queues:
            # scores(0,0,t=0) needs wq[j0]+xq[qb0]+bq and wk+xkv[kb0]
            nc.sync.dma_start(b_s[:, 0:DT], ball[:, 0:DT])          # bq (tiny)
            nc.scalar.dma_start(w_q[:, :, 0:128], wq_r[:, :, 0:128])
            nc.gpsimd.dma_start(w_k[:], wkt.rearrange("(o p) d -> p o d", p=128))
            nc.sync.dma_start(xq_s[:, :, 0:512], xq_r[:, :, 0:512])
            nc.scalar.dma_start(w_q[:, :, 128:512], wq_r[:, :, 128:512])
            nc.gpsimd.dma_start(xkv_s[:, :, 0:512], xkv_r[:, :, 0:512])
            nc.gpsimd.dma_start(w_v[:], wvt.rearrange("(o p) d -> p o d", p=128))
            nc.sync.dma_start(b_s[:, DT:], ball[:, DT:])            # bo | bv
            nc.scalar.dma_start(xkv_s[:, :, 1536:2048], xkv_r[:, :, 1536:2048])
            nc.gpsimd.dma_start(xkv_s[:, :, 512:1024], xkv_r[:, :, 512:1024])
            nc.gpsimd.dma_start(xkv_s[:, :, 1024:1536], xkv_r[:, :, 1024:1536])
            nc.sync.dma_start(xq_s[:, :, 512:1024], xq_r[:, :, 512:1024])
            nc.sync.dma_start(w_o[:], wot.rearrange("(o p) d -> p o d", p=128))

            # preload exp activation table early (no DMA dependency)
            nc.vector.memset(zz[:], 0.0)
            nc.vector.memset(lnc[:], LN_C)
            nc.scalar.activation(
                expw[:], zz[:], mybir.ActivationFunctionType.Exp
            )

            # ones columns of V (col 64 of each 65-wide head block)
            ones8 = pp.tile([128, H], F32)
            nc.gpsimd.memset(ones8[:], 1.0)
            for i in range(NTK):
                nc.gpsimd.tensor_copy(
                    v8[:, i, :].rearrange("p (h w) -> p h w", w=VW)[:, :, HD],
                    ones8[:],
                )

            with (
                tc.tile_pool(name="sp", bufs=3, space="PSUM") as sp,
                tc.tile_pool(name="op", bufs=1, space="PSUM") as opl,
                tc.tile_pool(name="e8", bufs=6) as e8p,
                tc.tile_pool(name="oc", bufs=2) as ocp,
                tc.tile_pool(name="rr", bufs=4) as rrp,
                tc.tile_pool(name="rb", bufs=4) as rbp,
                tc.tile_pool(name="on", bufs=4) as onp,
                tc.tile_pool(name="o64", bufs=2) as o64p,
                tc.tile_pool(name="yt", bufs=3) as yp,
            ):
                # ---------------- projections ----------------
                def q_proj(j, qb, drain="scalar"):
                    qsl = slice(qb * 512, (qb + 1) * 512)
                    ps = sp.tile([128, 2, 512], F32, tag="s2", name="s2t")
                    for k in range(KT):
                        nc.tensor.matmul(
                            ps[:, 0, :],
                            w_q[:, k, j * 128 : (j + 1) * 128],
                            xq_s[:, k, qsl],
                            start=(k == 0), stop=(k == KT - 1),
                        )
                    if drain == "scalar":
                        nc.scalar.activation(
                            qt[:, j, qsl], ps[:, 0, :],
                            mybir.ActivationFunctionType.Identity,
                            bias=bq_s[:, j : j + 1],
                        )
                    else:
                        nc.vector.tensor_scalar_add(
                            qt[:, j, qsl], ps[:, 0, :], bq_s[:, j : j + 1]
                        )

                def k_proj(j, c, drain="scalar"):
                    # K^T[j-tile, 512 keys] via fp8 DR (weights x64 on host,
                    # undone in the drain); no bias (softmax shift-invariant).
                    ksl = slice(c * 512, (c + 1) * 512)
                    ps = sp.tile([128, 2, 512], F32, tag="s2", name="s2t")
                    for g in range(KT // 2):
                        gs = slice(2 * g, 2 * g + 2)
                        nc.tensor.matmul(
                            ps[:, 0, :],
                            w_k[:, gs, j * 128 : (j + 1) * 128],
                            xkv_s[:, gs, ksl],
                            start=(g == 0), stop=(g == KT // 2 - 1),
                            perf_mode=mybir.MatmulPerfMode.DoubleRow,
                        )
                    if drain == "scalar":
                        nc.scalar.activation(
                            kt[:, j, ksl], ps[:, 0, :],
                            mybir.ActivationFunctionType.Copy,
                            scale=1.0 / W8SCALE,
                        )
                    else:
                        nc.vector.tensor_scalar_mul(
                            kt[:, j, ksl], ps[:, 0, :], 1.0 / W8SCALE
                        )

                def v_proj(blk):
                    # V natural [keys, feat] for one 128-key tile; f8 drain
                    # with bias on DVE.
                    isl = slice(blk * 128, (blk + 1) * 128)
                    ps = sp.tile([128, 2, 512], F32, tag="s2", name="s2t")
                    for g in range(KT // 2):
                        gs = slice(2 * g, 2 * g + 2)
                        nc.tensor.matmul(
                            ps[:, 0, :], xkv_s[:, gs, isl], w_v[:, gs, :],
                            start=(g == 0), stop=(g == KT // 2 - 1),
                            perf_mode=mybir.MatmulPerfMode.DoubleRow,
                        )
                    nc.vector.scalar_tensor_tensor(
                        v8[:, blk, :].rearrange("p (h w) -> p h w", w=VW)[:, :, 0:HD],
                        ps[:, 0, :].rearrange("p (h w) -> p h w", w=HD),
                        1.0 / W8SCALE,
                        bv_s.rearrange("p (h w) -> p h w", w=HD),
                        mybir.AluOpType.mult,
                        mybir.AluOpType.add,
                    )

                # ---------------- softmax normalize (deferred) ----------------
                pending_norm = []

                def norm_parity(hp, qb, oc, par):
                    # oc: SBUF copy of o psum [VW, 2, 512]; row 64 = r.
                    qsl = slice(qb * 512, (qb + 1) * 512)
                    rr2 = rrp.tile([1, 512], F32, name="rr2t")
                    nc.sync.dma_start(rr2[0:1, :], oc[64:65, par, :])
                    rr3 = rrp.tile([1, 512], F32, name="rr3t")
                    nc.vector.reciprocal_approx_fast(rr3[0:1, :], rr2[0:1, :])
                    rbc = rbp.tile([64, 512], F32)
                    nc.gpsimd.partition_broadcast(rbc[:], rr3[0:1, :])
                    on = onp.tile([64, 512], BF16)
                    nc.vector.tensor_tensor(
                        on[:], oc[0:64, par, :], rbc[:], mybir.AluOpType.mult
                    )
                    if par == 0:
                        nc.vector.tensor_tensor(
                            qt[0:64, hp, qsl], qt[0:64, hp, qsl], on[:],
                            mybir.AluOpType.add,
                        )
                    else:
                        on64 = o64p.tile([128, 512], BF16)
                        nc.sync.dma_start(on64[64:128, :], on[:])
                        nc.vector.tensor_tensor(
                            qt[64:128, hp, qsl], qt[64:128, hp, qsl],
                            on64[64:128, :], mybir.AluOpType.add,
                        )

                def flush_norm(idx):
                    while pending_norm and pending_norm[0][0] <= idx:
                        _, args = pending_norm.pop(0)
                        norm_parity(*args)

                # ---------------- attention ----------------
                def attn_block(hp, qb, extras=None, o_drain_eng="scalar",
                               pv_delay=None, tail_prev=None):
                    if pv_delay is None:
                        pv_delay = PV_DELAY
                    qsl = slice(qb * 512, (qb + 1) * 512)
                    h_e, h_o = 2 * hp, 2 * hp + 1
                    s_kps = SCALAR_KPS[2 * hp + qb]
                    o_e = opl.tile([VW, 512], F32, name="oe")
                    o_o = opl.tile([VW, 512], F32, name="oo")

                    e8_cur = None
                    e8_by_kp = {}

                    def pv(kp):
                        e8k = e8_by_kp.pop(kp)
                        ksl = slice(2 * kp, 2 * kp + 2)
                        nc.tensor.matmul(
                            o_e[:], v8[:, ksl, h_e * VW : (h_e + 1) * VW],
                            e8k[:, :, 0, :],
                            start=(kp == 0), stop=(kp == NKP - 1),
                            perf_mode=mybir.MatmulPerfMode.DoubleRow,
                        )
                        nc.tensor.matmul(
                            o_o[:], v8[:, ksl, h_o * VW : (h_o + 1) * VW],
                            e8k[:, :, 1, :],
                            start=(kp == 0), stop=(kp == NKP - 1),
                            perf_mode=mybir.MatmulPerfMode.DoubleRow,
                        )

                    for t in range(NTK):
                        flush_norm(t)
                        if extras and t in extras:
                            for fn in extras[t]:
                                fn()
                        isl = slice(t * 128, (t + 1) * 128)
                        s2 = sp.tile([128, 2, 512], F32, tag="s2", name="s2t")
                        nc.tensor.matmul(
                            s2[:, 0, :], kt[0:64, hp, isl], qt[0:64, hp, qsl],
                            start=True, stop=True,
                        )
                        nc.tensor.matmul(
                            s2[:, 1, :], kt[64:128, hp, isl], qt[64:128, hp, qsl],
                            start=True, stop=True,
                        )
                        kp, par = t // 2, t % 2
                        if par == 0:
                            e8_cur = e8p.tile([128, 2, 2, 512], F8, name="e8t")
                            e8_by_kp[kp] = e8_cur
                        if kp in s_kps:
                            nc.scalar.activation(
                                e8_cur[:, par, :, :], s2[:],
                                mybir.ActivationFunctionType.Exp,
                                bias=lnc[:, 0:1], scale=SCALE,
                            )
                        else:
                            nc.vector.tensor_scalar(
                                e8_cur[:, par, :, :].bitcast(U8), s2[:],
                                EXP_A8, EXP_B8,
                                mybir.AluOpType.mult, mybir.AluOpType.add,
                            )
                        # software pipeline: PV(kp) pv_delay iterations
                        # after its exps were issued
                        if t >= pv_delay and t % 2 == 1:
                            pv((t - pv_delay) // 2)

                    def tail():
                        # PV tail + o drain, deferred into the next block's
                        # second iteration so its first scores fill the PE
                        # while the last exps complete.
                        for kp in range((NTK - pv_delay) // 2 + 1, NKP):
                            pv(kp)
                        oc = ocp.tile([OC, 2, 512], F32, name="oct")
                        if O_BOTH_SCALAR:
                            nc.scalar.activation(
                                oc[:, 0, :], o_e[0:OC, :],
                                mybir.ActivationFunctionType.Copy
                            )
                            nc.scalar.activation(
                                oc[:, 1, :], o_o[0:OC, :],
                                mybir.ActivationFunctionType.Copy
                            )
                        elif o_drain_eng == "scalar":
                            nc.scalar.activation(
                                oc[:, 0, :], o_e[0:OC, :],
                                mybir.ActivationFunctionType.Copy
                            )
                            nc.vector.tensor_copy(oc[:, 1, :], o_o[0:OC, :])
                        else:
                            nc.vector.tensor_copy(oc[:, 0, :], o_e[0:OC, :])
                            nc.scalar.activation(
                                oc[:, 1, :], o_o[0:OC, :],
                                mybir.ActivationFunctionType.Copy
                            )
                        pending_norm.append((3, (hp, qb, oc, 0)))
                        pending_norm.append((8, (hp, qb, oc, 1)))
                    return tail

                # ---------------- phase 3 ----------------
                def phase3_mm(z, q, jj, ks):
                    qsl = slice(q * 512, (q + 1) * 512)
                    for half in range(2):
                        j = 2 * jj + half
                        for k in ks:
                            nc.tensor.matmul(
                                z[:, half, :],
                                w_o[:, k, j * 128 : (j + 1) * 128],
                                qt[:, k, qsl],
                                start=(k == 0), stop=(k == KT - 1),
                            )

                def phase3_drain(z, q, jj):
                    qsl = slice(q * 512, (q + 1) * 512)
                    for half in range(2):
                        j = 2 * jj + half
                        yt = yp.tile([128, 512], BF16, name="ytt")
                        nc.scalar.activation(
                            yt[:], z[:, half, :],
                            mybir.ActivationFunctionType.Relu,
                            bias=bo_s[:, j : j + 1],
                        )
                        yo = yp.tile([128, 512], BF16, name="yot")
                        nc.vector.tensor_tensor(
                            yo[:], yt[:], qt[:, j, qsl], mybir.AluOpType.add
                        )
                        nc.sync.dma_start(
                            outt[j * 128 : (j + 1) * 128, qsl], yo[:]
                        )

                # ---------------- schedule ----------------
                # minimal prelude, then block (0,0) self-feeds via extras
                # (PV delayed 5 iters there so JIT V projections keep up)
                q_proj(0, 0, drain="scalar")
                k_proj(0, 0, drain="scalar")
                ex00 = {
                    0: [lambda: q_proj(1, 0, "scalar")],
                    1: [lambda: q_proj(2, 0, "scalar")],
                    2: [lambda: q_proj(3, 0, "scalar"),
                        lambda: v_proj(0), lambda: v_proj(1)],
                    3: [lambda: v_proj(2), lambda: k_proj(0, 1, "scalar")],
                    4: [lambda: v_proj(3), lambda: v_proj(4)],
                    5: [lambda: v_proj(5)],
                    6: [lambda: v_proj(6), lambda: k_proj(0, 2, "scalar")],
                    7: [lambda: v_proj(7), lambda: v_proj(8)],
                    8: [lambda: v_proj(9)],
                    10: [lambda: v_proj(10), lambda: k_proj(0, 3, "scalar"),
                         lambda: v_proj(11)],
                    11: [lambda: v_proj(12)],
                    12: [lambda: v_proj(13)],
                    13: [lambda: v_proj(14), lambda: v_proj(15)],
                }
                attn_block(0, 0, extras=ex00, o_drain_eng="scalar",
                           pv_delay=5)()

                # Q(j0, qb1) must land before block (0,1) scores
                q_proj(0, 1, drain="scalar")
                ex01 = {
                    2: [lambda: k_proj(1, 0, "scalar")],
                    4: [lambda: q_proj(1, 1, "scalar"),
                        lambda: k_proj(1, 1, "scalar")],
                    7: [lambda: k_proj(1, 2, "scalar")],
                    9: [lambda: q_proj(2, 1, "scalar")],
                    11: [lambda: k_proj(1, 3, "scalar")],
                    13: [lambda: q_proj(3, 1, "scalar")],
                }
                attn_block(0, 1, extras=ex01, o_drain_eng="vector")()

                # K(j2) spread over blocks (1,0)/(1,1); K(j3) over (2,0)/(2,1)
                exk = lambda j, c, e: (lambda: k_proj(j, c, e))
                attn_block(1, 0, extras={
                    4: [exk(2, 0, "scalar")], 10: [exk(2, 1, "scalar")],
                }, o_drain_eng="scalar")()
                attn_block(1, 1, extras={
                    4: [exk(2, 2, "scalar")], 10: [exk(2, 3, "scalar")],
                }, o_drain_eng="vector")()
                attn_block(2, 0, extras={
                    4: [exk(3, 0, "scalar")], 10: [exk(3, 1, "scalar")],
                }, o_drain_eng="scalar")()
                attn_block(2, 1, extras={
                    4: [exk(3, 2, "scalar")], 10: [exk(3, 3, "scalar")],
                }, o_drain_eng="vector")()
                attn_block(3, 0, o_drain_eng="scalar")()
                attn_block(3, 1, o_drain_eng="vector")()

                # phase 3: qb0 first (norms already flushed); qb1
                # k=0..2 pre-flush, k=3 after the final norms.
                for jj in range(DT // 2):
                    z = sp.tile([128, 2, 512], F32, tag="s2", name="s2t")
                    phase3_mm(z, 0, jj, range(KT))
                    phase3_drain(z, 0, jj)
                z1 = [
                    sp.tile([128, 2, 512], F32, tag="s2", name="s2t")
                    for _ in range(DT // 2)
                ]
                for jj in range(DT // 2):
                    phase3_mm(z1[jj], 1, jj, range(KT - 1))
                flush_norm(NTK)
                for jj in range(DT // 2):
                    phase3_mm(z1[jj], 1, jj, [KT - 1])
                    phase3_drain(z1[jj], 1, jj)

    nc.compile()
    return nc


_NC = None


def _get_nc():
    global _NC
    if _NC is None:
        _NC = _build()
    return _NC


def kernel(**inputs) -> np.ndarray:
    bf = ml_dtypes.bfloat16
    f8 = ml_dtypes.float8_e4m3
    q = np.asarray(inputs["query"], dtype=np.float32)
    kv = np.asarray(inputs["key_value"], dtype=np.float32)
    shared = {
        "wqt": np.ascontiguousarray(np.asarray(inputs["Wq"], np.float32).T).astype(bf),
        "wkt": np.ascontiguousarray(np.asarray(inputs["Wk"], np.float32).T * W8SCALE).astype(f8),
        "wvt": np.ascontiguousarray(np.asarray(inputs["Wv"], np.float32).T * W8SCALE).astype(f8),
        "wot": np.ascontiguousarray(np.asarray(inputs["Wo"], np.float32).T).astype(bf),
        "ball": np.ascontiguousarray(np.concatenate(
            [
                np.asarray(inputs["bq"], np.float32).reshape(DT, 128).T,
                np.asarray(inputs["bo"], np.float32).reshape(DT, 128).T,
                np.broadcast_to(np.asarray(inputs["bv"], np.float32), (128, D)),
            ],
            axis=1,
        )),
    }
    in_maps = []
    for c in range(NCORES):
        b, half = divmod(c, 2)
        qs = q[b, half * TQ : (half + 1) * TQ]
        in_maps.append(
            {
                "xqt": np.ascontiguousarray(qs.T).astype(bf),
                "xkvt": np.ascontiguousarray(kv[b].T).astype(f8),
                **shared,
            }
        )

    nc = _get_nc()
    res = run_bass_kernel_spmd(nc, in_maps, core_ids=list(range(NCORES)))
    kernel._last_results = res  # for test harness introspection

    out = np.empty((B, SQ, D), np.float32)
    for c in range(NCORES):
        b, half = divmod(c, 2)
        out[b, half * TQ : (half + 1) * TQ] = res.results[c]["outt"].astype(np.float32).T
    return out
